# revision 22
# baseline (speedup 1.0000x reference)
"""2-layer GCN (PyG GCNConv x2 + relu + log_softmax) on 8 Trainium2 NeuronCores.

Strategy: shard destination nodes (and their incoming edges) across the 8
cores. Each layer:
  1. dense  h = x @ W  data-parallel over the core's node shard,
     scaled to g = dinv * h  (dinv = 1/sqrt(weighted in-degree + self loop))
  2. AllGather the g-shards into a replicated bf16 node table
  3. edge pass: dma_gather g[src] rows for the core's (dst-sorted, padded)
     edges, build selection matrices S^T[e, n] = w_e * (dst_local[e] == n)
     on the vector engine (bf16, 2x DVE mode), segment-sum via TensorE bf16
     matmuls accumulated in PSUM per 128-node destination tile
  4. epilogue out[n] = dinv[n] * (psum[n] + g[n]) (+relu / log_softmax)

All tables, gathered rows, selection matrices and matmuls are bf16 (PSUM
accumulation stays fp32); the rel-err budget (2e-2) dwarfs bf16 noise.
Table rows are laid out partition-major (row = p*T + t) so the staging
DMA writes 128 long contiguous runs instead of one 256B run per node.

Per-edge DMA-gather descriptors dominate, so padding is minimized with a
per-dst-tile variable tile count (KAs[t]/KBs[t], shared by all cores for
SPMD) and a vector bin-packing pass that assigns nodes to tiles under
per-tile (A-edges, B-edges) capacity caps.

Cross-phase overlap: the layer-2 dense matmul for tile t runs inside
layer-1's epilogue, and the layer-2 low-half table collective is emitted
into the GPSIMD instruction stream *between* layer-1 gather chunks (the
Pool queue is in-order, so emitting it at phase-E top would stall it
behind every remaining layer-1 gather).  The log_softmax tail is split
low/high the same way.  Gather chunks are small at stream edges to cut
pipeline fill/drain.

dma_gather uses int16 indices, so the node table is split in two halves of
25600/24576 rows; each dst-tile's edge list is partitioned into A-half /
B-half groups, each padded to a multiple of 128.
"""
import sys

sys.path.insert(0, "/opt/trn_rl_repo")

import numpy as np
import ml_dtypes

from concourse import bass, mybir, bacc
import concourse.tile as tile
from concourse.bass_utils import run_bass_kernel_spmd

N = 50000
E = 800000
INCH = 128
HID = 128
OUT = 64
OUTP = 2 * OUT            # g2 table row padded to 128 cols (256B bf16)
NCORES = 8
NSH = N // NCORES          # 6250 nodes per shard
P = 128
NT = (NSH + P - 1) // P    # 49 dst tiles per shard
NPAD = NT * P              # 6272
LOWN = 3200                # low-half padded positions per shard (25 tiles)
HIGHN = 3072               # high-half padded positions per shard (24 tiles)
LT = LOWN // P             # 25
HT = HIGHN // P            # 24
TRA = NCORES * LOWN        # 25600 rows in table A (int16-safe)
TRB = NCORES * HIGHN       # 24576 rows in table B
CH = 48                    # main gather chunk size in edge tiles
NCACHE = 120               # B-region selection matrices cached across layers
ORDT = list(range(LT, NT)) + list(range(LT))   # high dst tiles first

f32 = mybir.dt.float32
bf16 = mybir.dt.bfloat16
i16 = mybir.dt.int16
i32 = mybir.dt.int32
AF = mybir.ActivationFunctionType
ALU = mybir.AluOpType
bfnp = ml_dtypes.bfloat16

_PROGRAM_CACHE = {}


def _chunk_bounds(tp):
    """Chunk boundaries over tp tiles: small edge chunks, CH in the middle."""
    bounds = [0, min(12, tp)]
    while tp - bounds[-1] > CH + 28:
        bounds.append(bounds[-1] + CH)
    while tp - bounds[-1] > 14:
        bounds.append(bounds[-1] + 14)
    if bounds[-1] < tp:
        bounds.append(tp)
    return [b for i, b in enumerate(bounds) if i == 0 or b > bounds[i - 1]]


def _build_program(KAs, KBs, trace=False, collectives=True, skip=(),
                   with_bias=True):
    KAs = tuple(int(k) for k in KAs)
    KBs = tuple(int(k) for k in KBs)
    key = (KAs, KBs, collectives, tuple(skip), with_bias)
    if key in _PROGRAM_CACHE:
        return _PROGRAM_CACHE[key]

    TPA = sum(KAs)           # edge tiles in A region
    TPB = sum(KBs)
    TP = TPA + TPB
    EPAD = TP * P
    offA = [0] * NT
    offB = [0] * NT
    accA = accB = 0
    for t in ORDT:
        offA[t] = accA
        offB[t] = accB
        accA += KAs[t]
        accB += KBs[t]
    KAmax = max(KAs)
    KBmax = max(KBs)
    boundsA = _chunk_bounds(TPA)
    boundsB = _chunk_bounds(TPB)

    nc = bacc.Bacc("TRN2", target_bir_lowering=False, debug=False,
                   enable_asserts=True, num_devices=NCORES)

    # inputs
    xT_d = nc.dram_tensor("xT", [P, NPAD], bf16, kind="ExternalInput")
    idxw_d = nc.dram_tensor("idxw", [P, EPAD // 16], i16, kind="ExternalInput")
    dstl_d = nc.dram_tensor("dstlT", [P, TP], f32, kind="ExternalInput")
    wt_d = nc.dram_tensor("wT", [P, TP], f32, kind="ExternalInput")
    dinv_d = nc.dram_tensor("dinv", [P, NT], f32, kind="ExternalInput")
    w1_d = nc.dram_tensor("W1", [INCH, HID], bf16, kind="ExternalInput")
    w2_d = nc.dram_tensor("W2", [HID, OUT], bf16, kind="ExternalInput")
    b1_d = nc.dram_tensor("b1b", [P, HID], f32, kind="ExternalInput")
    b2_d = nc.dram_tensor("b2b", [P, OUT], f32, kind="ExternalInput")
    id_d = nc.dram_tensor("ident", [P, P], bf16, kind="ExternalInput")

    out_d = nc.dram_tensor("out", [P, NT * OUT], bf16, kind="ExternalOutput")

    # internal DRAM (bf16 tables; partition-major row order)
    g1_d = nc.dram_tensor("g1_shard", [NPAD, HID], bf16)
    g2_d = nc.dram_tensor("g2_shard", [NPAD, OUTP], bf16)
    t1a_d = nc.dram_tensor("table1a", [TRA, HID], bf16, addr_space="Shared")
    t1b_d = nc.dram_tensor("table1b", [TRB, HID], bf16, addr_space="Shared")
    t2a_d = nc.dram_tensor("table2a", [TRA, OUTP], bf16, addr_space="Shared")
    t2b_d = nc.dram_tensor("table2b", [TRB, OUTP], bf16, addr_space="Shared")

    groups = [list(range(NCORES))]

    def coll(dst_ap, src_ap, nrows):
        if collectives:
            nc.gpsimd.collective_compute(
                "AllGather", ALU.bypass, replica_groups=groups,
                ins=[src_ap], outs=[dst_ap])
        else:
            nc.gpsimd.dma_start(dst_ap[:nrows], src_ap)

    with tile.TileContext(nc) as tc:
        with tc.tile_pool(name="pers", bufs=1) as pers:
            # tiles + loads in first-use order (the sync DMA queue is FIFO)
            w1 = pers.tile([INCH, HID], bf16)
            nc.sync.dma_start(w1[:], w1_d[:])
            dinv = pers.tile([P, NT], f32)
            nc.sync.dma_start(dinv[:], dinv_d[:])
            xb_ctx = tc.tile_pool(name="xb", bufs=1)
            xbps_ctx = tc.tile_pool(name="xbps", bufs=2, space="PSUM")
            xb = xb_ctx.__enter__()
            xbps = xbps_ctx.__enter__()
            xT = xb.tile([P, NPAD], bf16)
            nc.sync.dma_start(xT[:, :LOWN], xT_d[:, :LOWN])
            nc.sync.dma_start(xT[:, LOWN:], xT_d[:, LOWN:])
            idxw = pers.tile([P, EPAD // 16], i16)
            dstlT = pers.tile([P, TP], f32)
            wT = pers.tile([P, TP], f32)
            w2 = pers.tile([HID, OUT], bf16)
            ident = pers.tile([P, P], bf16)
            if with_bias:
                b1b = pers.tile([P, HID], f32)
                nc.sync.dma_start(b1b[:], b1_d[:])
                b2b = pers.tile([P, OUT], f32)
                nc.sync.dma_start(b2b[:], b2_d[:])

            iota_i = pers.tile([P, P], i32, tag="iota_i")
            nc.gpsimd.iota(iota_i[:], pattern=[[1, P]], base=0,
                           channel_multiplier=0)
            cols_b = pers.tile([P, P], bf16)
            nc.vector.tensor_copy(cols_b[:], iota_i[:])

            g1_sb = pers.tile([P, NT * HID], bf16)
            relu1T = pers.tile([P, NPAD], bf16)
            g2_sb = pers.tile([P, NT * OUTP], bf16)
            o_sb = pers.tile([P, NT * OUT], bf16)
            negm_sb = pers.tile([P, NT], f32)
            se_sb = pers.tile([P, NT], f32)
            lse_sb = pers.tile([P, NT], f32)
            # zero once so the padded halves of each g2 row stay zero
            nc.gpsimd.memset(g2_sb[:], 0.0)

            # ---------- phase B: dense layer 1, g1 = dinv * (x @ W1) ----------
            for t in range(NT):
                ps = xbps.tile([P, HID], f32, tag="xps")
                nc.tensor.matmul(ps[:], lhsT=xT[:, t * P:(t + 1) * P],
                                 rhs=w1[:], start=True, stop=True)
                nc.scalar.activation(
                    g1_sb[:, t * HID:(t + 1) * HID], ps[:], AF.Copy,
                    scale=dinv[:, t:t + 1])
                if t == LT - 1:
                    # low half complete -> start its AllGather now; it
                    # overlaps the high-half dense compute + B collective
                    nc.sync.dma_start(
                        g1_d[:LOWN].rearrange("(p t) c -> p t c", p=P),
                        g1_sb[:, :LT * HID].rearrange(
                            "p (t c) -> p t c", c=HID))
                    coll(t1a_d[:], g1_d[:LOWN, :], LOWN)
            nc.sync.dma_start(
                g1_d[LOWN:].rearrange("(p t) c -> p t c", p=P),
                g1_sb[:, LT * HID:].rearrange("p (t c) -> p t c", c=HID))
            # edge-pass-only inputs load after the dense staging so the
            # first table collective isn't queued behind them; the first
            # chunks' index slices land first
            qA = (boundsA[1] - boundsA[0]) * 8
            qB = (boundsB[1] - boundsB[0]) * 8
            nc.sync.dma_start(idxw[:, :qA], idxw_d[:, :qA])
            nc.sync.dma_start(idxw[:, TPA * 8:TPA * 8 + qB],
                              idxw_d[:, TPA * 8:TPA * 8 + qB])
            nc.sync.dma_start(idxw[:, qA:TPA * 8], idxw_d[:, qA:TPA * 8])
            nc.sync.dma_start(idxw[:, TPA * 8 + qB:],
                              idxw_d[:, TPA * 8 + qB:])
            nc.sync.dma_start(dstlT[:], dstl_d[:])
            nc.sync.dma_start(wT[:], wt_d[:])
            nc.sync.dma_start(w2[:], w2_d[:])
            nc.sync.dma_start(ident[:], id_d[:])
            xbps_ctx.__exit__(None, None, None)
            xb_ctx.__exit__(None, None, None)

            # selection matrices for the last NCACHE B-region slots are
            # built once in layer 1 and reused by layer 2
            stc = pers.tile([P, max(NCACHE, 1) * P], bf16)
            gb_ctx = tc.tile_pool(name="gbuf", bufs=2)
            gbp = gb_ctx.__enter__()

            # ---------- edge pass helper ----------
            def edge_pass(tabA_d, tabB_d, g_sb, GW, CW, MW, first, epilogue,
                          border="AB", pre=None):
                """Gather+SpMM over all dst tiles; epilogue(t, psum_tile).

                CW = gathered row width (bf16 elements, 256B-aligned);
                MW = matmul rhs width; g_sb/GW = per-tile self-term table
                (added into PSUM via an identity matmul).  first=True builds
                the shared selection-matrix cache, False reuses it.
                """
                chunksA = [None] * (len(boundsA) - 1)
                chunksB = [None] * (len(boundsB) - 1)

                def chunk_of(bounds, j):
                    lo, hi = 0, len(bounds) - 1
                    while lo + 1 < hi:           # find c with bounds[c] <= j
                        mid = (lo + hi) // 2
                        if bounds[mid] <= j:
                            lo = mid
                        else:
                            hi = mid
                    return lo

                with (
                    tc.tile_pool(name="stbuf", bufs=3) as stp,
                    tc.tile_pool(name="pse", bufs=3, space="PSUM") as pse,
                ):
                    def chunk(region, c):
                        lst, bounds, coff = (
                            (chunksA, boundsA, 0) if region == 0 else
                            (chunksB, boundsB, TPA))
                        if lst[c] is None:
                            ct = bounds[c + 1] - bounds[c]
                            buf = gbp.tile([P, CH * CW], bf16,
                                           tag=f"g{region}{first}")
                            if "gather" in skip:
                                lst[c] = buf
                                return (buf, bounds[c])
                            half = tabA_d[:] if region == 0 else tabB_d[:]
                            col0 = (coff + bounds[c]) * (P // 16)
                            with tc.high_priority(offset=300):
                                nc.gpsimd.dma_gather(
                                    out_ap=buf[:, :ct * CW].rearrange(
                                        "p (k c) -> p k c", c=CW),
                                    in_ap=half,
                                    idxs_ap=idxw[:, col0:col0 + ct * (P // 16)],
                                    num_idxs=ct * P,
                                    num_idxs_reg=ct * P,
                                    elem_size=CW,
                                    single_packet=False,
                                )
                            lst[c] = buf
                        return (lst[c], bounds[c])

                    # warm the first-needed chunk, then let `pre` emit into
                    # the Pool stream (e.g. the other half's collective)
                    chunk(0 if border == "AB" else 1, 0)
                    if pre is not None:
                        pre()

                    for t in ORDT:
                        KA_t, KB_t = KAs[t], KBs[t]
                        ps = pse.tile([P, MW], f32, tag="pse")
                        stA = stp.tile([P, KAmax * P], bf16, tag="stA")
                        stB = stp.tile([P, KBmax * P], bf16, tag="stB")

                        def lhsB(k):
                            cpos = offB[t] + k - (TPB - NCACHE)
                            if cpos >= 0:
                                return stc[:, cpos * P:(cpos + 1) * P]
                            return stB[:, k * P:(k + 1) * P]

                        if "st" not in skip:
                            for k in range(KA_t):
                                j = offA[t] + k
                                nc.vector.tensor_scalar(
                                    out=stA[:, k * P:(k + 1) * P],
                                    in0=cols_b[:],
                                    scalar1=dstlT[:, j:j + 1],
                                    scalar2=wT[:, j:j + 1],
                                    op0=ALU.is_equal, op1=ALU.mult)
                            for k in range(KB_t):
                                j = TPA + offB[t] + k
                                cached = offB[t] + k >= TPB - NCACHE
                                if cached and not first:
                                    continue
                                nc.vector.tensor_scalar(
                                    out=lhsB(k),
                                    in0=cols_b[:],
                                    scalar1=dstlT[:, j:j + 1],
                                    scalar2=wT[:, j:j + 1],
                                    op0=ALU.is_equal, op1=ALU.mult)

                        def mmA(first_mm):
                            for k in range(KA_t):
                                j = offA[t] + k
                                buf, base = chunk(0, chunk_of(boundsA, j))
                                slot = j - base
                                if "mm" in skip: continue
                                nc.tensor.matmul(
                                    ps[:], lhsT=stA[:, k * P:(k + 1) * P],
                                    rhs=buf[:, slot * CW:slot * CW + MW],
                                    start=(first_mm and k == 0), stop=False)

                        def mmB(first_mm):
                            for k in range(KB_t):
                                j = offB[t] + k
                                buf, base = chunk(1, chunk_of(boundsB, j))
                                slot = j - base
                                if "mm" in skip: continue
                                nc.tensor.matmul(
                                    ps[:], lhsT=lhsB(k),
                                    rhs=buf[:, slot * CW:slot * CW + MW],
                                    start=(first_mm and k == 0), stop=False)

                        if border == "AB":
                            mmA(True)
                            mmB(False)
                        else:
                            mmB(True)
                            mmA(False)
                        if "mm" not in skip:
                            # self-loop term: psum += I @ g_tile
                            nc.tensor.matmul(
                                ps[:], lhsT=ident[:],
                                rhs=g_sb[:, t * GW:t * GW + MW],
                                start=False, stop=True)
                        epilogue(t, ps)

            # ---------- phase C: edge pass layer 1 (+ fused dense layer 2) ----
            with tc.tile_pool(name="ep1", bufs=3) as ep1, \
                 tc.tile_pool(name="pst", bufs=2, space="PSUM") as pst, \
                 tc.tile_pool(name="psd2", bufs=2, space="PSUM") as psd2:
                def epi1(t, ps):
                    tmp = ep1.tile([P, HID], bf16, tag="tmp1")
                    nc.scalar.activation(tmp[:], ps[:], AF.Copy,
                                         scale=dinv[:, t:t + 1])
                    if with_bias:
                        nc.vector.tensor_tensor(
                            out=tmp[:], in0=tmp[:], in1=b1b[:], op=ALU.add)
                    pt = pst.tile([P, P], bf16, tag="pst")
                    nc.tensor.transpose(pt[:], tmp[:], ident[:])
                    nc.scalar.activation(relu1T[:, t * P:(t + 1) * P], pt[:],
                                         AF.Relu)
                    # dense layer 2 for this tile, fused in so the layer-2
                    # tables are ready while layer 1 is still draining
                    ps2 = psd2.tile([P, OUT], f32, tag="psd2")
                    nc.tensor.matmul(ps2[:], lhsT=relu1T[:, t * P:(t + 1) * P],
                                     rhs=w2[:], start=True, stop=True)
                    nc.scalar.activation(
                        g2_sb[:, t * OUTP:t * OUTP + OUT], ps2[:], AF.Copy,
                        scale=dinv[:, t:t + 1])
                    # high tiles are processed first, so their table is
                    # staged mid-pass; the low half at the very end
                    if t == NT - 1:
                        nc.sync.dma_start(
                            g2_d[LOWN:].rearrange("(p t) c -> p t c", p=P),
                            g2_sb[:, LT * OUTP:].rearrange(
                                "p (t c) -> p t c", c=OUTP))
                        coll(t2b_d[:], g2_d[LOWN:, :], HIGHN)
                    if t == LT - 1:
                        nc.sync.dma_start(
                            g2_d[:LOWN].rearrange("(p t) c -> p t c", p=P),
                            g2_sb[:, :LT * OUTP].rearrange(
                                "p (t c) -> p t c", c=OUTP))
                        coll(t2a_d[:], g2_d[:LOWN, :], LOWN)

                edge_pass(t1a_d, t1b_d, g1_sb, HID, HID, HID, True, epi1,
                          border="AB",
                          pre=lambda: coll(t1b_d[:], g1_d[LOWN:, :], HIGHN))

            # ---------- phase E: edge pass layer 2 + log_softmax ----------
            def final_half(lo, hi):
                nc.scalar.activation(lse_sb[:, lo:hi], se_sb[:, lo:hi], AF.Ln)
                for t in range(lo, hi):
                    osl = o_sb[:, t * OUT:(t + 1) * OUT]
                    nc.vector.tensor_scalar(
                        out=osl, in0=osl,
                        scalar1=negm_sb[:, t:t + 1],
                        scalar2=lse_sb[:, t:t + 1],
                        op0=ALU.add, op1=ALU.subtract)
                nc.sync.dma_start(out_d[:, lo * OUT:hi * OUT],
                                  o_sb[:, lo * OUT:hi * OUT])

            with tc.tile_pool(name="ep2", bufs=3) as ep2:
                def epi2(t, ps):
                    osl = o_sb[:, t * OUT:(t + 1) * OUT]
                    nc.scalar.activation(osl, ps[:], AF.Copy,
                                         scale=dinv[:, t:t + 1])
                    if with_bias:
                        nc.vector.tensor_tensor(
                            out=osl, in0=osl, in1=b2b[:], op=ALU.add)
                    m = ep2.tile([P, 1], f32, tag="m")
                    nc.vector.tensor_reduce(
                        out=m[:], in_=osl, axis=mybir.AxisListType.X,
                        op=ALU.max)
                    nc.vector.tensor_scalar(
                        out=negm_sb[:, t:t + 1], in0=m[:], scalar1=-1.0,
                        scalar2=None, op0=ALU.mult)
                    ex = ep2.tile([P, OUT], bf16, tag="ex")
                    nc.scalar.activation(ex[:], osl, AF.Exp,
                                         bias=negm_sb[:, t:t + 1],
                                         accum_out=se_sb[:, t:t + 1])
                    if t == NT - 1:
                        final_half(LT, NT)
                    if t == 11:
                        final_half(0, 12)
                    if t == 18:
                        final_half(12, 19)

                edge_pass(t2a_d, t2b_d, g2_sb, OUTP, OUTP, OUT, False, epi2,
                          border="BA")
                final_half(19, LT)
            gb_ctx.__exit__(None, None, None)

    nc.compile()
    _PROGRAM_CACHE[key] = nc
    return nc


def _wrap_idx(lidx):
    """[EPAD] int -> [128, EPAD//16] int16 (16-partition wrap, 8x replicated)."""
    n = lidx.shape[0]
    w16 = lidx.reshape(n // 16, 16).T.astype(np.int16)   # [16, n/16]
    return np.ascontiguousarray(np.tile(w16, (8, 1)))


def _pack_nodes(aw, bw, capA, capB, capN):
    """Assign nodes (per-node A/B in-edge counts aw/bw) to len(capA) tiles.

    Greedy (largest node first) picking the tile whose max fill ratio across
    A-edges / B-edges / node-slots stays lowest, under hard caps; returns a
    list of node-index lists.  Raises RuntimeError when a node fits nowhere.
    """
    remA = capA.astype(np.float64).copy()
    remB = capB.astype(np.float64).copy()
    remN = capN.astype(np.float64).copy()
    cA = np.maximum(capA, 1).astype(np.float64)
    cB = np.maximum(capB, 1).astype(np.float64)
    cN = np.maximum(capN, 1).astype(np.float64)
    buckets = [[] for _ in range(len(capA))]
    order = np.argsort(-(aw + bw), kind="stable")
    for i in order:
        a, b = aw[i], bw[i]
        fit = (remN > 0) & (remA >= a) & (remB >= b)
        if not fit.any():
            raise RuntimeError("packing failed")
        fillA = 1.0 - (remA - a) / cA
        fillB = 1.0 - (remB - b) / cB
        fillN = 1.0 - (remN - 1) / cN
        score = np.where(fit, np.maximum(np.maximum(fillA, fillB), fillN),
                         np.inf)
        tsel = int(np.argmin(score))
        buckets[tsel].append(i)
        remA[tsel] -= a
        remB[tsel] -= b
        remN[tsel] -= 1
    return buckets


def _profile(need_edges, ntiles):
    """Per-tile K profile covering need_edges, extras on the first tiles."""
    need = int(np.ceil(need_edges / P))
    base = need // ntiles
    Ks = np.full(ntiles, base, np.int64)
    Ks[:need - base * ntiles] += 1
    return Ks


def _prep_inputs(x, edge_index, edge_weight):
    src = np.asarray(edge_index[0], dtype=np.int64)
    dst = np.asarray(edge_index[1], dtype=np.int64)
    w = np.asarray(edge_weight, dtype=np.float32)
    x = np.asarray(x, dtype=np.float32)

    deg = np.bincount(dst, weights=w.astype(np.float64), minlength=N)
    deg = deg.astype(np.float32) + 1.0
    dinv = (1.0 / np.sqrt(deg)).astype(np.float32)

    shard_src = src // NSH
    shard_dst = dst // NSH
    # half = src's ORIGINAL local id >= LOWN (permutation-independent; the
    # packer keeps low nodes in low tiles).
    halfe = ((src % NSH) >= LOWN).astype(np.int64)

    # Per-node in-edge counts by src half.
    nodeA = np.zeros(N, np.int64)
    nodeB = np.zeros(N, np.int64)
    np.add.at(nodeA, dst[halfe == 0], 1)
    np.add.at(nodeB, dst[halfe == 1], 1)

    halves = [(0, LOWN, np.arange(0, LT)), (LOWN, NSH, np.arange(LT, NT))]
    capN_t = np.full(NT, P, np.int64)
    capN_t[NT - 1] = NSH - (NT - 1) * P     # 106

    nodeAs = nodeA.reshape(NCORES, NSH)
    nodeBs = nodeB.reshape(NCORES, NSH)
    KAs = np.zeros(NT, np.int64)
    KBs = np.zeros(NT, np.int64)
    slack = 128
    while True:
        for lo, hi, tl in halves:
            needA = max(nodeAs[:, lo:hi].sum(axis=1).max() + slack, P)
            needB = max(nodeBs[:, lo:hi].sum(axis=1).max() + slack, P)
            KAs[tl] = _profile(needA, len(tl))
            KBs[tl] = _profile(needB, len(tl))
        capA_t = KAs * P
        capB_t = KBs * P
        try:
            perms = []
            iperms = np.empty((NCORES, NSH), np.int64)
            for s in range(NCORES):
                perm = np.empty(NSH, np.int64)
                posc = 0
                for lo, hi, tl in halves:
                    buckets = _pack_nodes(
                        nodeAs[s, lo:hi], nodeBs[s, lo:hi],
                        capA_t[tl], capB_t[tl], capN_t[tl])
                    for nodes in buckets:
                        nodes = np.asarray(nodes, np.int64) + lo
                        perm[posc:posc + len(nodes)] = nodes
                        posc += len(nodes)
                assert posc == NSH
                perms.append(perm)
                iperms[s][perm] = np.arange(NSH)
            break
        except RuntimeError:
            slack *= 2
            if slack > 65536:
                raise
    _prep_inputs.last_perms = perms

    TPA = int(KAs.sum())
    TPB = int(KBs.sum())
    # slot offsets follow the tile PROCESSING order (high tiles first)
    offA = np.zeros(NT + 1, np.int64)
    offB = np.zeros(NT + 1, np.int64)
    accA = accB = 0
    for t in ORDT:
        offA[t] = accA
        offB[t] = accB
        accA += KAs[t]
        accB += KBs[t]

    per_core = []
    for s in range(NCORES):
        m = shard_dst == s
        es = src[m]
        ew = w[m]
        half = halfe[m]
        edp = iperms[s][dst[m] - s * NSH]      # permuted local dst
        t = edp >> 7
        pos = iperms[shard_src[m], es % NSH]   # permuted position of src
        # table row within its half-table, partition-major:
        # A half: row = shard*LOWN + p*LT + t   (pos = t*128+p, t < LT)
        # B half: row = shard*HIGHN + p*HT + (t-LT)
        pt_ = pos >> 7
        pp_ = pos & 127
        trow = np.where(pos < LOWN,
                        shard_src[m] * LOWN + pp_ * LT + pt_,
                        shard_src[m] * HIGHN + pp_ * HT + (pt_ - LT))
        key = (half * NT + t)
        order = np.argsort(key, kind="stable")
        per_core.append((trow[order], edp[order], ew[order],
                         t[order], half[order]))

    in_maps = []
    for s in range(NCORES):
        es, ed, ew, t, half = per_core[s]
        lidx_all = np.zeros((TPA + TPB, P), np.int64)
        dstl_all = np.zeros((TPA + TPB, P), np.float32)
        w_all = np.zeros((TPA + TPB, P), np.float32)
        for h, Ks, off, base in ((0, KAs, offA, 0), (1, KBs, offB, TPA)):
            hm = half == h
            eh, edh, ewh, th = es[hm], ed[hm], ew[hm], t[hm]
            for tt in ORDT:
                K = int(Ks[tt])
                tm = th == tt
                cnt = int(tm.sum())
                assert cnt <= K * P, (s, h, tt, cnt, K * P)
                row = base + off[tt]
                flat_l = np.zeros(K * P, np.int64)
                flat_d = np.zeros(K * P, np.float32)
                flat_w = np.zeros(K * P, np.float32)
                flat_l[:cnt] = eh[tm]
                flat_d[:cnt] = (edh[tm] & 127).astype(np.float32)
                flat_w[:cnt] = ewh[tm]
                lidx_all[row:row + K] = flat_l.reshape(K, P)
                dstl_all[row:row + K] = flat_d.reshape(K, P)
                w_all[row:row + K] = flat_w.reshape(K, P)

        xs = x[s * NSH + perms[s]]
        xT = np.zeros((P, NPAD), bfnp)
        xT[:, :NSH] = xs.T.astype(bfnp)
        full = np.ones(NPAD, np.float32)
        full[:NSH] = dinv[s * NSH + perms[s]]
        dv = np.ascontiguousarray(full.reshape(NT, P).T)

        in_maps.append({
            "xT": xT,
            "idxw": _wrap_idx(lidx_all.reshape(-1)),
            "dstlT": np.ascontiguousarray(dstl_all.T),
            "wT": np.ascontiguousarray(w_all.T),
            "dinv": dv,
        })
    return in_maps, tuple(int(k) for k in KAs), tuple(int(k) for k in KBs)


def kernel(x, edge_index, edge_weight, W1, b1, W2, b2, trace=False):
    in_maps, KAs, KBs = _prep_inputs(x, edge_index, edge_weight)
    shared = {
        "W1": np.asarray(W1, np.float32).astype(bfnp),
        "W2": np.asarray(W2, np.float32).astype(bfnp),
        "b1b": np.tile(np.asarray(b1, np.float32)[None, :], (P, 1)),
        "b2b": np.tile(np.asarray(b2, np.float32)[None, :], (P, 1)),
        "ident": np.eye(P, dtype=bfnp),
    }
    for im in in_maps:
        im.update(shared)

    with_bias = bool(np.any(shared["b1b"]) or np.any(shared["b2b"]))
    nc = _build_program(KAs, KBs, with_bias=with_bias)
    res = run_bass_kernel_spmd(nc, in_maps, core_ids=list(range(NCORES)),
                               trace=trace)
    perms = _prep_inputs.last_perms
    out = np.empty((N, OUT), np.float32)
    for s in range(NCORES):
        o = res.results[s]["out"].astype(np.float32).reshape(P, NT, OUT)
        shard = np.ascontiguousarray(o.transpose(1, 0, 2)).reshape(NPAD, OUT)
        out[s * NSH + perms[s]] = shard[:NSH]
    kernel.last_results = res
    return out
